# revision 27
# baseline (speedup 1.0000x reference)
"""LocalWindowMultiHeadAttention on 8 Trainium2 NeuronCores via Bass/Tile.

x [1,128,128,128] f32, 8 heads x head_dim 16, 7x7 window (radius 3), reflect
padding, 128x128 Wq/Wk/Wv/Wo projections.

Sharding: H split into 8 shards of 16 rows; each core receives its shard plus
a 3-row halo on each side cut from the reflect-padded image (no device-to-
device exchange). Tensors cross the wire in bf16; weights/masks are
call-invariant and cached on device by content hash.

Per-core device kernel (channel-major planes [C=128 part, pix]):
  - GpSimd repacks x into a tile-major query plane and 8 overlapping 22-col
    key/value strips (walrus matmul APs must be 1-D free).
  - PE projects Q (3 head-planes, heads at 32-aligned partition bases),
    K (3 head-planes), V; scale 1/sqrt(hd) folded into Wq on host. Biases:
    bk/per-query terms cancel in softmax, bq asserted zero, bv/bo folded
    into a host-side output bias.
  - 16 query tiles of 8x16 px; the union of their 7x7 windows is 14x22 keys,
    split into 3 row-chunks of 110 keys; per chunk only the query rows that
    can see those key rows are scored (272 instead of 384 score cols).
  - Scores on PE (K=16), exp fused over head pairs on ACT, multiplicative
    0/1 window mask on DVE, AV via [maskedexp]^T @ [V|1] PSUM-accumulated
    (denominator rides along as the 17th column), one reciprocal+broadcast
    multiply per tile, PE transpose back, O projection, DMA out.
"""

import json
from contextlib import ExitStack

import ml_dtypes
import numpy as np

import concourse.bass as bass
import concourse.tile as tile
from concourse import mybir
from concourse.bass_utils import run_bass_kernel_spmd

F32 = np.float32
BF16 = ml_dtypes.bfloat16

RADIUS = 3
H = W = C = 128
NH, HD = 8, 16
RPC = 16                 # center rows per core
PR, PC = 22, 134         # padded slice dims per core
NPIX = PR * PC           # 2948
BH, BW = 8, 16           # query tile (rows x cols)
NSTR = W // BW           # 8 column strips
SPIX = PR * 22           # 484 strip pixels
# chunks: (key rows r0:r1, valid query cols q0:q1, col offset in the packed
# 272-wide score block). Emitted middle-chunk-first so its AV matmul
# (start=True, all 128 queries) resets PSUM before the partial chunks
# accumulate. The last chunk re-reads key row 9 (mask zeroes it) so every
# chunk is exactly 110 keys.
CHS = ((5, 10, 0, 128, 0), (0, 5, 0, 80, 128), (9, 14, 64, 128, 208))
CHV = ((0, 5), (5, 10), (9, 14))   # vaug build order -> kv chunk index
KVIX = (1, 0, 2)                   # CHS index -> CHV chunk index
SCW = 272                          # packed score cols per head
N_CORES = 8

_AF = mybir.ActivationFunctionType


def _split_multiwait_drains(bir_bytes: bytes) -> bytes:
    """This container's walrus accepts at most ONE sem wait per instruction.
    Hoist extra waits onto preceding same-engine Drain carriers."""
    m = json.loads(bir_bytes)
    for f in m["functions"]:
        for blk in f["blocks"]:
            new = []
            for inst in blk.get("instructions", []):
                si = inst.get("sync_info")
                ow = (si or {}).get("on_wait") or []
                if len(ow) > 1:
                    for j, wt in enumerate(ow[:-1]):
                        new.append({
                            "name": inst["name"] + f"-w{j}",
                            "opcode": "Drain",
                            "engine": inst.get("engine", "SP"),
                            "debug": inst.get("debug", 0),
                            "is_reset_sema": False,
                            "ins": [], "outs": [],
                            "sync_info": {"on_update": [], "on_wait": [wt]},
                        })
                    si["on_wait"] = [ow[-1]]
                new.append(inst)
            blk["instructions"] = new
    return json.dumps(m).encode()


def build_nc() -> bass.Bass:
    nc = bass.Bass("TRN2", target_bir_lowering=False, debug=False)
    bf = mybir.dt.bfloat16

    d_xsT = nc.declare_dram_parameter("xsT", [128, NPIX], bf, isOutput=False)
    d_wq = nc.declare_dram_parameter("wqT", [128, 384], bf, isOutput=False)
    d_wk = nc.declare_dram_parameter("wkT", [128, 384], bf, isOutput=False)
    d_wv = nc.declare_dram_parameter("wvT", [128, 128], bf, isOutput=False)
    d_wo = nc.declare_dram_parameter("woT", [128, 128], bf, isOutput=False)
    d_mask = nc.declare_dram_parameter("mask", [128, SCW], bf, isOutput=False)
    d_id = nc.declare_dram_parameter("ident", [128, 128], bf, isOutput=False)
    d_out = nc.declare_dram_parameter("outT", [128, RPC * W], bf, isOutput=True)

    with tile.TileContext(nc) as tc:
        with ExitStack() as ctx:
            consts = ctx.enter_context(tc.tile_pool(name="consts", bufs=1))
            planes = ctx.enter_context(tc.tile_pool(name="planes", bufs=1))

            t_xsT = consts.tile([128, NPIX], bf)
            for dq in range(4):
                a = (NPIX // 4) * dq
                b = NPIX if dq == 3 else (NPIX // 4) * (dq + 1)
                nc.sync.dma_start(t_xsT[:, a:b], d_xsT[:, a:b])
            t_wq = consts.tile([128, 384], bf)
            nc.sync.dma_start(t_wq[:], d_wq[:])
            t_wk = consts.tile([128, 384], bf)
            nc.sync.dma_start(t_wk[:], d_wk[:])
            t_wv = consts.tile([128, 128], bf)
            nc.sync.dma_start(t_wv[:], d_wv[:])
            t_wo = consts.tile([128, 128], bf)
            nc.sync.dma_start(t_wo[:], d_wo[:])
            t_mask = consts.tile([128, SCW], bf)
            nc.sync.dma_start(t_mask[:], d_mask[:])
            t_id = consts.tile([128, 128], bf)
            nc.sync.dma_start(t_id[:], d_id[:])

            x3 = t_xsT.rearrange("p (r c) -> p r c", c=PC)

            # x repacked tile-major (queries) and strip-major (keys/values)
            # on GpSimd; walrus matmul operand APs must be 1-D free.
            t_xq = planes.tile([128, RPC * W], bf)
            for trow in range(RPC // BH):
                dst = t_xq[:, 1024 * trow : 1024 * (trow + 1)].rearrange(
                    "p (tc r c) -> p tc r c", r=BH, c=BW
                )
                src = x3[:, 3 + BH * trow : 3 + BH * trow + BH, 3 : 3 + W
                         ].rearrange("p r (tc c) -> p tc r c", c=BW)
                nc.gpsimd.tensor_copy(dst, src)
            t_xs = planes.tile([128, NSTR * SPIX], bf)
            xs3 = t_xs.rearrange("p (s r c) -> p s r c", r=PR, c=22)
            for s in range(NSTR):
                nc.gpsimd.tensor_copy(
                    xs3[:, s, :, :], x3[:, :, BW * s : BW * s + 22]
                )

            t_QT0 = planes.tile([128, RPC * W], bf)
            t_QT1 = planes.tile([128, RPC * W], bf)
            t_QT2 = planes.tile([128, RPC * W], bf)
            t_KS0 = planes.tile([128, NSTR * SPIX], bf)
            t_KS1 = planes.tile([128, NSTR * SPIX], bf)
            t_KS2 = planes.tile([128, NSTR * SPIX], bf)
            t_VS = planes.tile([128, NSTR * SPIX], bf)
            t_aoT0 = planes.tile([128, RPC * W // 2], bf)
            t_aoT1 = planes.tile([128, RPC * W // 2], bf)
            t_outT = planes.tile([128, RPC * W], bf)
            qtm = [t_QT0, t_QT1, t_QT2]
            kst = [t_KS0, t_KS1, t_KS2]

            # ---- projections ----
            with tc.tile_pool(name="psP", bufs=3, space="PSUM") as psP:
                def _pcopy(k, dst, src_ap):
                    if k % 2 == 0:
                        nc.scalar.activation(dst, src_ap, _AF.Copy)
                    else:
                        nc.vector.tensor_copy(dst, src_ap)

                ncopy = 0
                for P, qplane in enumerate(qtm):
                    for i in range(4):
                        ps = psP.tile([128, 512], mybir.dt.float32, tag="pp")
                        nc.tensor.matmul(
                            ps[:], t_wq[:, 128 * P : 128 * P + 128],
                            t_xq[:, 512 * i : 512 * (i + 1)],
                            start=True, stop=True,
                        )
                        _pcopy(ncopy, qplane[:, 512 * i : 512 * (i + 1)], ps[:])
                        ncopy += 1
                for s in range(NSTR):
                    for P in range(3):
                        ps = psP.tile([128, 512], mybir.dt.float32, tag="pp")
                        nc.tensor.matmul(
                            ps[:, :SPIX], t_wk[:, 128 * P : 128 * P + 128],
                            t_xs[:, SPIX * s : SPIX * (s + 1)],
                            start=True, stop=True,
                        )
                        _pcopy(ncopy, kst[P][:, SPIX * s : SPIX * (s + 1)],
                               ps[:, :SPIX])
                        ncopy += 1
                    ps = psP.tile([128, 512], mybir.dt.float32, tag="pp")
                    nc.tensor.matmul(
                        ps[:, :SPIX], t_wv[:],
                        t_xs[:, SPIX * s : SPIX * (s + 1)],
                        start=True, stop=True,
                    )
                    _pcopy(ncopy, t_VS[:, SPIX * s : SPIX * (s + 1)],
                           ps[:, :SPIX])
                    ncopy += 1

            a3p = [t_aoT0.rearrange("p (r c) -> p r c", c=W),
                   t_aoT1.rearrange("p (r c) -> p r c", c=W)]

            # ---- attention, 16 query tiles ----
            with ExitStack() as actx:
                p_s = actx.enter_context(tc.tile_pool(name="ps_s", bufs=2, space="PSUM"))
                p_vt = actx.enter_context(tc.tile_pool(name="ps_vt", bufs=1, space="PSUM"))
                p_av = actx.enter_context(tc.tile_pool(name="ps_av", bufs=2, space="PSUM"))
                p_tr = actx.enter_context(tc.tile_pool(name="ps_tr", bufs=1, space="PSUM"))
                sb = actx.enter_context(tc.tile_pool(name="sb_work", bufs=3))
                sbat = actx.enter_context(tc.tile_pool(name="sb_at", bufs=6))

                for trow in range(RPC // BH):
                    for tcol in range(NSTR):
                        r_off = BH * trow
                        t_idx = trow * NSTR + tcol
                        q0t = 128 * t_idx
                        # V union chunks -> pixel-major [keys, C], one fused
                        # PSUM tile, one DVE copy, ones col for denominators
                        pvt = p_vt.tile([128, 384], bf, tag="pvt")
                        for kv, (r0, r1) in enumerate(CHV):
                            a0 = SPIX * tcol + 22 * (r_off + r0)
                            nc.tensor.transpose(
                                pvt[:110, 128 * kv : 128 * kv + 128],
                                t_VS[:, a0 : a0 + 110], t_id[:],
                            )
                        vaug = sb.tile([128, 3 * NH * (HD + 1)], bf, tag="vaug")
                        vv = vaug.rearrange("p (k h x) -> p k h x", h=NH, x=HD + 1)
                        nc.vector.tensor_copy(
                            vv[:110, :, :, 0:HD],
                            pvt.rearrange("p (k h d) -> p k h d", h=NH, d=HD)[:110],
                        )
                        nc.vector.memset(vv[:110, :, :, HD : HD + 1], 1.0)

                        ao = sb.tile([128, 128], bf, tag="ao")
                        pav = p_av.tile([128, NH * (HD + 1)], mybir.dt.float32, tag="av")
                        pavv = pav.rearrange("p (h x) -> p h x", x=HD + 1)
                        for hp2 in range(NH // 2):
                            # two heads share one PSUM score tile and one exp
                            ps = p_s.tile([128, 1024], mybir.dt.float32, tag="s")
                            At = sbat.tile([128, 2 * SCW], bf, tag="At")
                            for hh in range(2):
                                h = 2 * hp2 + hh
                                P, slot = divmod(h, 3)
                                sp = 32 * slot
                                for (r0, r1, q0, q1, co) in CHS:
                                    a0 = SPIX * tcol + 22 * (r_off + r0)
                                    nc.tensor.matmul(
                                        ps[:110, 512 * hh + co : 512 * hh + co + (q1 - q0)],
                                        kst[P][sp : sp + HD, a0 : a0 + 110],
                                        qtm[P][sp : sp + HD, q0t + q0 : q0t + q1],
                                        start=True, stop=True,
                                    )
                            nc.scalar.activation(
                                At.rearrange("p (h x) -> p h x", x=SCW)[:110],
                                ps.rearrange("p (h x) -> p h x", x=512)[:110, :, 0:SCW],
                                _AF.Exp,
                            )
                            mask_eng = (
                                nc.gpsimd if trow == 0
                                else (nc.vector if (t_idx + hp2) % 2 == 0 else nc.gpsimd)
                            )
                            mask_eng.tensor_mul(
                                At.rearrange("p (h x) -> p h x", x=SCW)[:110],
                                At.rearrange("p (h x) -> p h x", x=SCW)[:110],
                                t_mask[:110].rearrange("p (h x) -> p h x", h=1)
                                .broadcast_to([110, 2, SCW]),
                            )
                            for hh in range(2):
                                h = 2 * hp2 + hh
                                for ci, (r0, r1, q0, q1, co) in enumerate(CHS):
                                    nc.tensor.matmul(
                                        pavv[q0:q1, h, :],
                                        At[:110, SCW * hh + co : SCW * hh + co + (q1 - q0)],
                                        vv[:110, KVIX[ci], h, :],
                                        start=(ci == 0), stop=(ci == 2),
                                        skip_group_check=True,
                                    )
                        rec = sb.tile([128, NH], mybir.dt.float32, tag="rec")
                        rec3 = rec.rearrange("p (h x) -> p h x", x=1)
                        nc.vector.reciprocal(rec3[:], pavv[:, :, HD : HD + 1])
                        nc.vector.tensor_mul(
                            ao.rearrange("p (h d) -> p h d", d=HD)[:],
                            pavv[:, :, 0:HD],
                            rec3.broadcast_to([128, NH, HD]),
                        )
                        ptr = p_tr.tile([128, 128], bf, tag="tr")
                        nc.tensor.transpose(ptr[:], ao[:], t_id[:])
                        nc.vector.tensor_copy(
                            a3p[trow][:, 0:BH, BW * tcol : BW * tcol + BW],
                            ptr.rearrange("p (r c) -> p r c", r=BH),
                        )

            # ---- O projection (per trow half, starts after its 8 tiles) ----
            with tc.tile_pool(name="psO", bufs=2, space="PSUM") as psO:
                for i in range(4):
                    src_ao = (t_aoT0, t_aoT1)[i // 2]
                    ps = psO.tile([128, 512], mybir.dt.float32, tag="po")
                    nc.tensor.matmul(
                        ps[:], t_wo[:],
                        src_ao[:, 512 * (i % 2) : 512 * (i % 2 + 1)],
                        start=True, stop=True,
                    )
                    nc.scalar.activation(
                        t_outT[:, 512 * i : 512 * (i + 1)], ps[:], _AF.Copy
                    )
                    nc.sync.dma_start(
                        d_out[:, 512 * i : 512 * (i + 1)],
                        t_outT[:, 512 * i : 512 * (i + 1)],
                    )

    _orig = nc.to_json_bytes
    nc.to_json_bytes = lambda: _split_multiwait_drains(_orig())
    return nc


def _window_mask() -> np.ndarray:
    """[128, 272] packed mask: per chunk, rows = 110 union keys, cols = the
    query range that can see those key rows (query index q = qr*16+qc)."""
    m = np.zeros((128, SCW), F32)
    for (r0, r1, q0, q1, co) in CHS:
        q = np.arange(q0, q1)
        qr, qc = q // BW, q % BW
        j = np.arange(110)
        kr, kc = r0 + j // 22, j % 22
        dr = kr[:, None] - qr[None, :]
        dc = kc[:, None] - qc[None, :]
        valid = (dr >= 0) & (dr <= 6) & (dc >= 0) & (dc <= 6)
        if r0 == 9:
            valid &= kr[:, None] >= 10  # row 9 already scored in chunk 2
        m[:110, co : co + (q1 - q0)] = valid
    return m.astype(BF16)


def _head_planes(Wt: np.ndarray) -> np.ndarray:
    """[128 cin, 128 cout] -> [128, 384]: head h at cols 128*(h//3)+32*(h%3).
    SBUF AP base partitions are limited to {0,32,64}, so 3 heads per plane."""
    out = np.zeros((128, 384), F32)
    for h in range(NH):
        P, s = divmod(h, 3)
        out[:, 128 * P + 32 * s : 128 * P + 32 * s + HD] = Wt[:, HD * h : HD * h + HD]
    return np.ascontiguousarray(out).astype(BF16)


_CACHE: dict = {}


def _get_nc() -> bass.Bass:
    if "nc" not in _CACHE:
        _CACHE["nc"] = build_nc()
    return _CACHE["nc"]


def make_in_maps(x, Wq, bq, Wk, bk, Wv, bv, Wo, bo) -> list[dict]:
    # Softmax is invariant to per-query additive score terms, so bk cancels
    # entirely. bq only contributes a key-side term scale*(Wk_h^T bq_h); the
    # problem pins bq = 0, which we assert. bv passes through the softmax
    # (weights sum to 1) and bo is additive, so both fold into a host-side
    # output bias bo_eff = Wo @ bv + bo.
    assert np.abs(np.asarray(bq, F32)).max() == 0.0, "nonzero bq unsupported"
    scale = F32(1.0 / np.sqrt(HD))
    xp = np.pad(
        np.asarray(x, F32)[0], ((RADIUS, RADIUS), (RADIUS, RADIUS), (0, 0)),
        mode="reflect",
    )  # [134, 134, 128]
    const = {
        "wqT": _head_planes(np.asarray(Wq, F32).T * scale),
        "wkT": _head_planes(np.asarray(Wk, F32).T),
        "wvT": np.ascontiguousarray(np.asarray(Wv, F32).T).astype(BF16),
        "woT": np.ascontiguousarray(np.asarray(Wo, F32).T).astype(BF16),
        "mask": _window_mask(),
        "ident": np.eye(128, dtype=BF16),
    }
    in_maps = []
    for i in range(N_CORES):
        xs = xp[RPC * i : RPC * i + PR]  # [22, 134, 128]
        xsT = np.ascontiguousarray(xs.reshape(NPIX, 128).T.astype(BF16))
        in_maps.append({**const, "xsT": xsT})
    return in_maps


def run_spmd(in_maps, trace=False, **kw):
    return run_bass_kernel_spmd(
        _get_nc(), in_maps, list(range(N_CORES)), trace=trace, **kw
    )


class _Runner:
    """Build the PJRT executable once; keep device-resident input buffers
    keyed by content hash; chain output-buffer donation across calls."""

    def __init__(self):
        import jax
        from jax.experimental.shard_map import shard_map
        from jax.sharding import Mesh, PartitionSpec
        from concourse import bass2jax, mybir as mb

        bass2jax.install_neuronx_cc_hook()
        self.jax = jax
        nc = _get_nc()
        partition_name = (
            nc.partition_id_tensor.name if nc.partition_id_tensor else None
        )
        in_names, out_names, out_avals, zero_shapes = [], [], [], []
        for alloc in nc.m.functions[0].allocations:
            if not isinstance(alloc, mb.MemoryLocationSet):
                continue
            name = alloc.memorylocations[0].name
            if alloc.kind == "ExternalInput":
                if name != partition_name:
                    in_names.append(name)
            elif alloc.kind == "ExternalOutput":
                shape = tuple(alloc.tensor_shape)
                dtype = mb.dt.np(alloc.dtype)
                out_names.append(name)
                out_avals.append(jax.core.ShapedArray(shape, dtype))
                zero_shapes.append((shape, dtype))
        self.in_names, self.out_names = in_names, out_names
        self.zero_shapes = zero_shapes
        n_params, n_outs = len(in_names), len(out_names)
        donate = tuple(range(n_params, n_params + n_outs))

        def _body(*args):
            operands = list(args)
            if partition_name is not None:
                operands.append(bass2jax.partition_id_tensor())
            outs = bass2jax._bass_exec_p.bind(
                *operands,
                out_avals=tuple(out_avals),
                in_names=tuple(in_names + out_names
                               + ([partition_name] if partition_name else [])),
                out_names=tuple(out_names),
                lowering_input_output_aliases=(),
                sim_require_finite=True,
                sim_require_nnan=True,
                nc=nc,
            )
            return tuple(outs)

        devices = jax.devices()[:N_CORES]
        mesh = Mesh(np.asarray(devices), ("core",))
        spec = PartitionSpec("core")
        self.sharding = jax.sharding.NamedSharding(mesh, spec)
        self.sharded = jax.jit(
            shard_map(
                _body, mesh=mesh,
                in_specs=(spec,) * (n_params + n_outs),
                out_specs=(spec,) * n_outs,
                check_rep=False,
            ),
            donate_argnums=donate,
            keep_unused=True,
        )
        self._dev_cache: dict = {}   # name -> (digest, jax.Array)
        self._donate_next = None     # list of output arrays to donate

    def _put(self, name: str, arr: np.ndarray):
        import hashlib

        digest = hashlib.blake2b(arr.tobytes(), digest_size=16).digest()
        hit = self._dev_cache.get(name)
        if hit is not None and hit[0] == digest:
            return hit[1]
        darr = self.jax.device_put(arr, self.sharding)
        self._dev_cache[name] = (digest, darr)
        return darr

    def __call__(self, in_maps):
        concat = {
            name: np.concatenate([m[name] for m in in_maps], axis=0)
            for name in self.in_names
        }
        args = [self._put(name, concat[name]) for name in self.in_names]
        if self._donate_next is None:
            outs = [
                np.zeros((N_CORES * s[0], *s[1:]), d)
                for (s, d) in self.zero_shapes
            ]
        else:
            outs = self._donate_next
        res = self.sharded(*args, *outs)
        host = [np.asarray(r) for r in res]
        self._donate_next = list(res)
        return [
            {
                name: host[i].reshape(N_CORES, *self.zero_shapes[i][0])[c]
                for i, name in enumerate(self.out_names)
            }
            for c in range(N_CORES)
        ]


def _get_runner() -> "_Runner":
    if "runner" not in _CACHE:
        _CACHE["runner"] = _Runner()
    return _CACHE["runner"]


def assemble_output(results, bo_eff=None) -> np.ndarray:
    h = np.stack([np.asarray(results[i]["outT"]) for i in range(N_CORES)])
    out = (
        h.reshape(N_CORES, C, RPC, W).transpose(0, 2, 3, 1)
        .reshape(1, H, W, C).astype(F32)
    )
    if bo_eff is not None:
        out += bo_eff
    return out


def kernel(x, Wq, bq, Wk, bk, Wv, bv, Wo, bo) -> np.ndarray:
    in_maps = make_in_maps(x, Wq, bq, Wk, bk, Wv, bv, Wo, bo)
    bo_eff = (
        np.asarray(Wo, F32) @ np.asarray(bv, F32) + np.asarray(bo, F32)
    ).astype(F32)
    results = _get_runner()(in_maps)
    return assemble_output(results, bo_eff)
